# revision 31
# baseline (speedup 1.0000x reference)
"""LocallyConnected2d (B=8, C_in=32, 48x48, C_out=32, 3x3, pad 1) on 8 trn2 cores.

Strategy: shard the spatial-location axis L = H*W across cores (6 image rows
each). Weight streaming dominates -> memory-bound; weights are quantized to
fp8 e3m4 (global scale sw, folded into the fp16 x operand so PSUM holds y
directly) halving the dominant HBM traffic vs fp16.

PE instruction count is the secondary bottleneck (~15 ns/instr issue rate), so
each matmul covers FOUR locations: stationary = x-patch view [97, 32]
((kw*32+c)+ones row, (4 locs, 8 batch)), moving = W tile [97, 128]
((4 locs, 32 out)). Row 96 of the kh=0 W tile carries bias/sw (ones row in x
is sw), so bias accumulates for free. Outputs land in PSUM [32j.., (li', o)]
with only the li==li' diagonal blocks valid; the fp16 out tile ships garbage
too (still 4x less than useful fp32) and the host takes the diagonal.

DMA plan: the 16 DMA engines are shared by all queues (~240 GB/s aggregate),
and HWDGE queues crawl while the gpsimd SWDGE queue streams — so all bulk
reads go on gpsimd in exact consumption order (x rows 0-2, W groups 0-1,
x row 3, W groups 2..17 in 5 chunks), late-needed x rows 4-7 ride the SP
queue, and the per-macro out tiles ship on the Act queue. Weights live in
ONE [97, 27648] SBUF tile (bias row = one [1, 27648B] transfer); the x ones
row is a 1/16 memset with bias pre-scaled by 16 on host. Partition counts
stay at 96/128 (divisible by 16) — a 97-row DMA runs on a single engine.
"""

import numpy as np

import concourse.bacc as bacc
import concourse.tile as tile
from concourse import mybir
from concourse.bass_utils import run_bass_kernel_spmd

B, C_IN, H, W = 8, 32, 48, 48
C_OUT = 32
N_CORES = 8
RP = H // N_CORES  # rows per core (6)
LP = RP * W  # locations per core (288)
NGRP = LP // 16  # 16-loc groups per core (18)
MACROS = [(0, 2), (2, 4), (6, 4), (10, 4), (14, 3), (17, 1)]  # (first group, n)
XA = 3 * W * B  # stat rows 0-2 (1152 elems), needed by macro 0
XB1 = 4 * W * B  # row 3 boundary (1536)

DT16 = True
F16 = mybir.dt.float16
F8 = mybir.dt.float8e3  # e3m4
F32 = mybir.dt.float32
NPF8 = mybir.dt.np(F8)
F8MAX = 15.5  # e3m4 max normal
XF = (RP + 2) * W * B  # x3 free size (3072)

_nc = None


def _build():
    nc = bacc.Bacc(
        "TRN2", target_bir_lowering=False, debug=False, num_devices=N_CORES
    )
    statA = nc.dram_tensor("statA", [96, XA], F16, kind="ExternalInput")
    statB1 = nc.dram_tensor("statB1", [96, XB1 - XA], F16, kind="ExternalInput")
    statB2 = nc.dram_tensor("statB2", [96, XF - XB1], F16, kind="ExternalInput")
    wds = [
        nc.dram_tensor(f"w{i}", [96, ng * 1536], F8, kind="ExternalInput")
        for i, (_, ng) in enumerate(MACROS)
    ]
    wb = nc.dram_tensor("wb", [1, NGRP * 1536], F8, kind="ExternalInput")
    out = nc.dram_tensor("out", [128, NGRP * 128], F16, kind="ExternalOutput")

    with tile.TileContext(nc) as tc:
        with (
            tc.tile_pool(name="xpool", bufs=1) as xpool,
            tc.tile_pool(name="wpool", bufs=1) as wpool,
            tc.tile_pool(name="opool", bufs=1) as opool,
            tc.tile_pool(name="pspool", bufs=3, space="PSUM") as pspool,
        ):
            stat_sb = xpool.tile([97, XF], F16, tag="stat")
            wt = wpool.tile([97, NGRP * 1536], F8, tag="wt")
            out_sb = opool.tile([128, NGRP * 128], F16)

            # The gpsimd SWDGE queue streams at full rate; HWDGE queues crawl
            # while it runs. So ALL startup-critical + bulk reads go on gpsimd
            # in consumption order; only late-needed statB2 rides the (slow
            # under contention) SP queue, and writes go out on the Act queue.
            # ones row on the otherwise-idle DVE: a [1, N] memset is
            # single-lane (~2us) and would stall gpsimd's DMA issue there
            nc.vector.memset(stat_sb[96:97, :], 1.0 / 16.0)
            # sync's preamble ends first, so statB2 lands before the gpsimd
            # stream ramps; statA/statB1 lead the gpsimd queue in consumption
            # order. Anything transferring DURING the stream would crawl AND
            # steal ~15% of its rate.
            nc.scalar.dma_start(wt[96:97, :], wb[:, :])
            nc.sync.dma_start(stat_sb[0:96, XB1:XF], statB2[:, :])
            nc.gpsimd.dma_start(stat_sb[0:96, 0:XA], statA[:, :])
            nc.gpsimd.dma_start(wt[0:96, 0 : 2 * 1536], wds[0][:, :])
            nc.gpsimd.dma_start(stat_sb[0:96, XA:XB1], statB1[:, :])
            for mg, (g0, ng) in enumerate(MACROS[1:], start=1):
                nc.gpsimd.dma_start(
                    wt[0:96, g0 * 1536 : (g0 + ng) * 1536], wds[mg][:, :]
                )

            for mg, (g0, ng) in enumerate(MACROS):
                ps = pspool.tile([128, 512], F32, tag="ps")
                for gl in range(ng):
                    gi = g0 + gl
                    r, qg = divmod(gi, 3)
                    for kh in range(3):
                        for j in range(4):
                            q0 = qg * 16 + 4 * j
                            off = ((r + kh) * W + q0) * B
                            base = gi * 1536 + (kh * 4 + j) * 128
                            nc.tensor.matmul(
                                ps[32 * j : 32 * j + 32, gl * 128 : gl * 128 + 128],
                                stat_sb[0:97, off : off + 32],
                                wt[0:97, base : base + 128],
                                start=(kh == 0),
                                stop=(kh == 2),
                                skip_group_check=True,
                                tile_position=(0, 32 * j),
                            )
                nc.vector.tensor_copy(
                    out_sb[0:128, g0 * 128 : (g0 + ng) * 128],
                    ps[0:128, 0 : ng * 128],
                )
                # final macro's out goes on the idle SP queue so it does not
                # wait behind the previous out's HWDGE generation on Act
                oeng = nc.sync if mg == len(MACROS) - 1 else nc.scalar
                oeng.dma_start(
                    out[:, g0 * 128 : (g0 + ng) * 128],
                    out_sb[0:128, g0 * 128 : (g0 + ng) * 128],
                )
    nc.compile()
    return nc


def _shard(inputs):
    x = np.asarray(inputs["x"], np.float32)
    weight = np.asarray(inputs["weight"], np.float32)[0]
    bias = np.asarray(inputs["bias"], np.float32)[0]
    sw = max(float(np.abs(weight).max()) / F8MAX, 1e-20)

    xp = np.pad(x, ((0, 0), (0, 0), (1, 1), (1, 1)))  # (b, c, 50, 50)
    bias_t = bias.reshape(C_OUT, H * W).T  # (L, C_OUT)
    # (c, kh, kw, l, o) -> rows (kw*32+c)
    wperm = (
        weight.reshape(C_IN, 3, 3, H * W, C_OUT)
        .transpose(2, 0, 1, 3, 4)
        .reshape(96, 3, H * W, C_OUT)
    )

    in_maps = []
    for k in range(N_CORES):
        r0 = RP * k
        l0 = LP * k

        x3h = np.empty((3, C_IN, RP + 2, W, B), np.float32)
        for kw in range(3):
            x3h[kw] = xp[:, :, r0 : r0 + RP + 2, kw : kw + W].transpose(1, 2, 3, 0)
        x3h = (x3h.reshape(96, XF) * sw).astype(np.float16)
        m = {
            "statA": np.ascontiguousarray(x3h[:, 0:XA]),
            "statB1": np.ascontiguousarray(x3h[:, XA:XB1]),
            "statB2": np.ascontiguousarray(x3h[:, XB1:XF]),
        }
        wball = np.zeros((1, NGRP * 1536), np.float32)
        for mg, (g0, ng) in enumerate(MACROS):
            wm = np.zeros((96, ng * 1536), np.float32)
            for gl in range(ng):
                gi = g0 + gl
                r, qg = divmod(gi, 3)
                for kh in range(3):
                    for j in range(4):
                        lp0 = l0 + r * W + qg * 16 + 4 * j
                        base = (gl * 3 * 4 + kh * 4 + j) * 128
                        blk = wperm[:, kh, lp0 : lp0 + 4, :]  # (96, 4, 32)
                        wm[:, base : base + 128] = blk.reshape(96, 128) / sw
                        if kh == 0:
                            # ones row is a 1/16 memset, so ship bias*16
                            wball[0, gi * 1536 + j * 128 : gi * 1536 + (j + 1) * 128] = (
                                bias_t[lp0 : lp0 + 4, :].reshape(128) * 16.0
                            )
            m[f"w{mg}"] = wm.astype(NPF8)
        m["wb"] = wball.astype(NPF8)
        in_maps.append(m)
    return in_maps


def _get_nc():
    global _nc
    if _nc is None:
        _nc = _build()
    return _nc


def _gather(results):
    # out row p = 32j + 8li + b, col = gi*128 + 32li' + o; useful iff li==li'
    y = np.empty((B, C_OUT, H, W), np.float32)
    for k in range(N_CORES):
        arr = np.asarray(results[k]["out"], np.float32).reshape(4, 4, 8, NGRP, 4, 32)
        d = np.diagonal(arr, axis1=1, axis2=4)  # (j, b, gi, o, li)
        # y[b, o, r, qg, j, li] with gi = r*3 + qg
        d = d.transpose(1, 3, 2, 0, 4).reshape(B, C_OUT, RP, 3, 4, 4)
        y[:, :, RP * k : RP * (k + 1), :] = d.reshape(B, C_OUT, RP, W)
    return y


def kernel(**inputs):
    nc = _get_nc()
    res = run_bass_kernel_spmd(nc, _shard(inputs), list(range(N_CORES)))
    return _gather(res.results)
